# revision 5
# baseline (speedup 1.0000x reference)
"""Trainium2 Bass kernel for nn_LstmClassifier: batch-sharded LSTM over 8 cores.

fp8 (e4m3) DoubleRow recurrence:
    h is quantized to q = fp8(16*h) each step; gates = Wc8 . q via DoubleRow
    matmuls (2 k-tiles per instruction, 2x fp16 throughput). PSUM accumulates
    16*(Wc.h + bc); bias rides as a "ones-pair": q-tiles 4:6 hold 1.0 in
    partition 0, and Wc8 k-tiles 4:5 hold fp8(16*bc) hi/lo rows. The ACT
    engine applies sigmoid/tanh with scale=1/16 on whole gate tensors
    [128, 4, 256] (gate-major PSUM layout), so no per-partition bias fusion
    is needed. Step 0 additionally applies an fp8 lo-correction (q0lo) since
    h0 = relu(x.Win) is large and single-fp8 would inject too much noise.
    Cell math and the fp8 quantize run on DVE; tanh(c) on ACT; out-proj stays
    fp16 (exact) with bout added during the PSUM->SBUF eviction.
"""
import sys
import types
import numpy as np
import ml_dtypes

sys.path.insert(0, "/opt/trn_rl_repo")

import concourse.bass as bass  # noqa: E402
import concourse.tile as tile  # noqa: E402
from concourse import bacc, mybir  # noqa: E402
from concourse.bass_utils import run_bass_kernel_spmd  # noqa: E402

B, IN_DIM, HID, OUT_DIM, T = 2048, 1024, 512, 256, 64
NCORES = 8
BSH = B // NCORES          # 256 batch rows per core
KH = HID // 128            # 4 hidden k-tiles
KI = IN_DIM // 128         # 8 input k-tiles
F32 = mybir.dt.float32
F16 = mybir.dt.float16
F8 = mybir.dt.float8e4
AF = mybir.ActivationFunctionType
ALU = mybir.AluOpType
DR = mybir.MatmulPerfMode.DoubleRow
NPF8 = ml_dtypes.float8_e4m3fn

LAST_EXEC_NS = None


def _install_ntff_hook():
    try:
        import antenv.axon_hooks  # noqa: F401
        return True
    except ImportError:
        pass
    try:
        if "/root/.axon_site" not in sys.path:
            sys.path.insert(0, "/root/.axon_site")
        from trn_agent_boot.trn_boot import _ntff_profile_via_ctypes
        hook = _ntff_profile_via_ctypes("/opt/axon/libaxon_pjrt.so")
        if hook is None:
            return False
        import antenv
        mod = types.ModuleType("antenv.axon_hooks")
        mod._hook = hook
        mod.get_axon_ntff_profile_hook = lambda: mod._hook
        mod.set_axon_ntff_profile_hook = lambda h: setattr(mod, "_hook", h)
        antenv.axon_hooks = mod
        sys.modules["antenv.axon_hooks"] = mod
        return True
    except Exception:
        return False


def build_program(steps=T):
    nc = bacc.Bacc("TRN2", target_bir_lowering=False, debug=False)

    xT_d = nc.dram_tensor("xT", [128, KI, BSH], F16, kind="ExternalInput").ap()
    winT_d = nc.dram_tensor("winT", [128, KI, HID], F16, kind="ExternalInput").ap()
    bin_d = nc.dram_tensor("bin", [128, KH], F32, kind="ExternalInput").ap()
    # k-tiles 0:4 = fp8(Wc_hi) columns gate-major; 4:6 = bias hi/lo rows (x16)
    wc8_d = nc.dram_tensor("wc8", [128, 6, 4 * HID], F8, kind="ExternalInput").ap()
    wc8lo_d = nc.dram_tensor("wc8lo", [128, KH, 4 * HID], F8, kind="ExternalInput").ap()
    woutT_d = nc.dram_tensor("woutT", [128, KH, OUT_DIM], F16, kind="ExternalInput").ap()
    bout_d = nc.dram_tensor("bout", [128, OUT_DIM], F32, kind="ExternalInput").ap()
    out_d = nc.dram_tensor("out", [BSH, steps, OUT_DIM], F32, kind="ExternalOutput").ap()
    # [p, m, t, o]: batch row = m*128 + p
    out_v = out_d.rearrange("(m p) t o -> p m t o", p=128)

    from contextlib import ExitStack
    with tile.TileContext(nc) as tc, ExitStack() as ctx:
        consts = ctx.enter_context(tc.tile_pool(name="consts", bufs=1))
        qpool = ctx.enter_context(tc.tile_pool(name="q", bufs=3))
        qdpool = ctx.enter_context(tc.tile_pool(name="qd", bufs=2))
        hpool = ctx.enter_context(tc.tile_pool(name="h", bufs=3))
        cpool = ctx.enter_context(tc.tile_pool(name="c", bufs=3))
        sigp = ctx.enter_context(tc.tile_pool(name="sigs", bufs=8))
        tcp = ctx.enter_context(tc.tile_pool(name="tcp", bufs=4))
        osbp = ctx.enter_context(tc.tile_pool(name="osb", bufs=3))
        pg_pool = ctx.enter_context(tc.tile_pool(name="pgates", bufs=3, space="PSUM"))
        po_pool = ctx.enter_context(tc.tile_pool(name="pout", bufs=2, space="PSUM"))

        # ---- load constants ----
        xT = consts.tile([128, KI, BSH], F16)
        nc.sync.dma_start(xT[:], xT_d[:])
        winT = consts.tile([128, KI, HID], F16)
        nc.sync.dma_start(winT[:], winT_d[:])
        bin_sb = consts.tile([128, KH], F32)
        nc.sync.dma_start(bin_sb[:], bin_d[:])
        wc8 = consts.tile([128, 6, 4 * HID], F8)
        nc.sync.dma_start(wc8[:], wc8_d[:])
        wc8lo = consts.tile([128, KH, 4 * HID], F8)
        nc.sync.dma_start(wc8lo[:], wc8lo_d[:])
        woutT = consts.tile([128, KH, OUT_DIM], F16)
        nc.sync.dma_start(woutT[:], woutT_d[:])
        bout_sb = consts.tile([128, OUT_DIM], F32)
        nc.sync.dma_start(bout_sb[:], bout_d[:])
        bout_b = bout_sb[:].unsqueeze(1).broadcast_to([128, 2, OUT_DIM])

        # q buffers: [128, 6, BSH] fp8; tiles 4:6 are the bias "ones-pair"
        # (1.0 in partition 0, 0 elsewhere), preset once per rotating buffer.
        qtiles_init = []
        for r in range(3):
            qt = qpool.tile([128, 6, BSH], F8, tag="q", name=f"qinit{r}")
            nc.vector.memset(qt[:, 4:6, :], 0.0)
            nc.vector.memset(qt[0:1, 4:6, :], 1.0)
            qtiles_init.append(qt)

        # ---- input projection: h0 = relu(W_in @ x.T + b_in) (j-major) ----
        h = hpool.tile([128, KH, BSH], F16, tag="h")
        ph0 = pg_pool.tile([128, KH, BSH], F32, tag="pg", name="ph0")
        for jp in range(2):
            for k in range(KI):
                for j in (2 * jp, 2 * jp + 1):
                    nc.tensor.matmul(
                        ph0[:, j, :],
                        winT[:, k, j * 128:(j + 1) * 128],
                        xT[:, k, :],
                        start=(k == 0 and j % 2 == 0),
                        stop=(k == KI - 1 and j % 2 == 1),
                    )
        for j in range(KH):
            nc.scalar.activation(
                h[:, j, :], ph0[:, j, :], AF.Relu, bias=bin_sb[:, j:j + 1]
            )

        # q0 = fp8(16*h0); q0lo = fp8(16*(h0 - q0/16)) (step-0 lo correction)
        q = qtiles_init[0]
        nc.vector.tensor_scalar_mul(q[:, 0:4, :], h[:], 16.0)
        negr0 = consts.tile([128, KH, BSH], F16)
        nc.vector.scalar_tensor_tensor(
            negr0[:], q[:, 0:4, :], 0.0625, h[:], ALU.mult, ALU.subtract)
        q0lo = consts.tile([128, KH, BSH], F8)
        nc.vector.tensor_scalar_mul(q0lo[:], negr0[:], -16.0)
        qd = qdpool.tile([128, KH, BSH], F8, tag="qd", name="qd0")
        nc.vector.tensor_scalar_mul(qd[:], h[:], 1.0)

        WLO_FIRST = 10
        qinit_idx = 1

        def emit_gate_mms(pg, G, qt, extras=()):
            # one accumulation group per PSUM bank (j-pair)
            # extras: list of (lhsT_full, rhs_full) fp8 pairs, contracted over
            # k-tiles 0:4 in kp pairs
            for jp in range(2):
                mms = []
                for kp in range(3):  # kp 2 = ones/bias pair
                    for j in (2 * jp, 2 * jp + 1):
                        mms.append((None, kp, j))
                for ei, (lhs_t, rhs_t) in enumerate(extras):
                    for kp in range(2):
                        for j in (2 * jp, 2 * jp + 1):
                            mms.append((ei, kp, j))
                for idx, (ei, kp, j) in enumerate(mms):
                    mt = G * 4 + j
                    if ei is None:
                        lhs = wc8[:, 2 * kp:2 * kp + 2, mt * 128:(mt + 1) * 128]
                        rhs = qt[:, 2 * kp:2 * kp + 2, :]
                    else:
                        lhs_t, rhs_t = extras[ei]
                        lhs = lhs_t[:, 2 * kp:2 * kp + 2, mt * 128:(mt + 1) * 128]
                        rhs = rhs_t[:, 2 * kp:2 * kp + 2, :]
                    nc.tensor.matmul(
                        pg[:, j, :], lhs, rhs,
                        start=(idx == 0), stop=(idx == len(mms) - 1),
                        perf_mode=DR,
                    )

        def emit_outproj(h_src, t_idx):
            po = po_pool.tile([128, 2, OUT_DIM], F32, tag="po", name=f"po{t_idx}")
            for k in range(KH):
                for m in range(2):
                    nc.tensor.matmul(
                        po[:, m, :],
                        h_src[:, k, m * 128:(m + 1) * 128],
                        woutT[:, k, :],
                        start=(k == 0 and m == 0),
                        stop=(k == KH - 1 and m == 1),
                    )
            osb = osbp.tile([128, 2, OUT_DIM], F32, tag="osb", name=f"osb{t_idx}")
            nc.vector.tensor_add(osb[:], po[:], bout_b)
            nc.sync.dma_start(out_v[:, :, t_idx, :], osb[:])

        c = None
        for t in range(steps):
            extras = []
            if t == 0:
                extras.append((wc8, q0lo))
            if t < WLO_FIRST:
                extras.append((wc8lo, qd))
            # ---- gates (gate-major): i, f, g, o ----
            pgs = []
            sigs = []
            for G in range(4):
                pg = pg_pool.tile([128, KH, BSH], F32, tag="pg", name=f"pg{t}_{G}")
                emit_gate_mms(pg, G, q, extras=extras)
                pgs.append(pg)
                if t >= 1 and G == 1:
                    # out-proj for t-1 rides between gate mms
                    emit_outproj(h, t - 1)
                func = AF.Tanh if G == 2 else AF.Sigmoid
                sg = sigp.tile([128, KH, BSH], F16, tag=f"s{G}", name=f"s{t}_{G}")
                if G == 3:
                    # split o-gate act by j-pair: earlier h/q availability
                    nc.scalar.activation(sg[:, 0:2, :], pg[:, 0:2, :], func, scale=0.0625)
                    nc.scalar.activation(sg[:, 2:4, :], pg[:, 2:4, :], func, scale=0.0625)
                else:
                    nc.scalar.activation(sg[:], pg[:], func, scale=0.0625)
                sigs.append(sg)
            si, sf, tg, so = sigs

            # ---- cell update (DVE), split by j-pair ----
            c_new = cpool.tile([128, KH, BSH], F16, tag="c")
            t1 = sigp.tile([128, KH, BSH], F16, tag="t1", name=f"t1_{t}")
            tc_t = tcp.tile([128, KH, BSH], F16, tag="tc", name=f"tc{t}")
            h_new = hpool.tile([128, KH, BSH], F16, tag="h")
            q_new = (qtiles_init[qinit_idx] if qinit_idx is not None and t < 2
                     else qpool.tile([128, 6, BSH], F8, tag="q", name=f"q{t}"))
            if t < 2:
                qinit_idx = 2 if qinit_idx == 1 else None
            qd_new = None
            if t + 1 < WLO_FIRST:
                qd_new = qdpool.tile([128, KH, BSH], F8, tag="qd", name=f"qd{t+1}")
            for jp in range(2):
                s = slice(2 * jp, 2 * jp + 2)
                nc.vector.tensor_mul(t1[:, s, :], si[:, s, :], tg[:, s, :])
                if t == 0:
                    # c0 = 0 -> c1 = i*g
                    nc.vector.tensor_copy(c_new[:, s, :], t1[:, s, :])
                else:
                    nc.vector.tensor_mul(c_new[:, s, :], sf[:, s, :], c[:, s, :])
                    nc.vector.tensor_add(c_new[:, s, :], c_new[:, s, :], t1[:, s, :])
                # tanh(c) on ACT
                nc.scalar.activation(tc_t[:, s, :], c_new[:, s, :], AF.Tanh)
                nc.vector.tensor_mul(h_new[:, s, :], so[:, s, :], tc_t[:, s, :])
                nc.vector.tensor_scalar_mul(q_new[:, s, :], h_new[:, s, :], 16.0)
                if qd_new is not None:
                    nc.vector.tensor_scalar_mul(qd_new[:, s, :], h_new[:, s, :], 1.0)

            if qd_new is not None:
                qd = qd_new
            h = h_new
            c = c_new
            q = q_new
        emit_outproj(h, steps - 1)

    nc.compile()
    return nc


_PROGRAM = None


def _get_program():
    global _PROGRAM
    if _PROGRAM is None:
        _PROGRAM = build_program()
    return _PROGRAM


def _pack_inputs(x, W_in, b_in, W_ih, b_ih, W_hh, b_hh, W_out, b_out):
    f16, f32 = np.float16, np.float32
    Wc = np.asarray(W_ih, f32) + np.asarray(W_hh, f32)
    bc = (np.asarray(b_ih, f32) + np.asarray(b_hh, f32))
    # gate-major m-tiles: mt = G*4 + j, G in (i, f, g, o) = PyTorch row order
    perm = np.concatenate([
        np.arange(G * HID + j * 128, G * HID + j * 128 + 128)
        for G in range(4) for j in range(KH)
    ])
    Wc_r = Wc[perm]                      # [2048, 512]
    bc_r = bc[perm]                      # [2048]

    # wc8: [128, 6, 2048]: k-tiles 0:4 = fp8(Wc_r.T), 4:6 = bias hi/lo rows
    wc8 = np.zeros((128, 6, 4 * HID), NPF8)
    wc8[:, 0:4, :] = np.ascontiguousarray(
        Wc_r.T.reshape(KH, 128, 4 * HID).transpose(1, 0, 2)).astype(NPF8)
    bhi8 = (16.0 * bc_r).astype(NPF8)
    bres = 16.0 * bc_r - bhi8.astype(f32)
    wc8[0, 4, :] = bhi8
    wc8[0, 5, :] = bres.astype(NPF8)
    # wc8lo: fp8(16*(Wc - fp8(Wc))), contracted against qd = fp8(h) (scale 1)
    wcT_f32 = np.ascontiguousarray(
        Wc_r.T.reshape(KH, 128, 4 * HID).transpose(1, 0, 2))
    wc8lo = (16.0 * (wcT_f32 - wc8[:, 0:4, :].astype(f32))).astype(NPF8)

    winT = np.ascontiguousarray(
        np.asarray(W_in, f32).T.reshape(KI, 128, HID).transpose(1, 0, 2).astype(f16))
    woutT = np.ascontiguousarray(
        np.asarray(W_out, f32).T.reshape(KH, 128, OUT_DIM).transpose(1, 0, 2).astype(f16))
    bin_p = np.ascontiguousarray(np.asarray(b_in, f32).reshape(KH, 128).T)
    bout_p = np.ascontiguousarray(np.broadcast_to(np.asarray(b_out, f32), (128, OUT_DIM)))

    shared = {
        "winT": winT, "wc8": wc8, "wc8lo": wc8lo, "woutT": woutT,
        "bin": bin_p, "bout": bout_p,
    }
    in_maps = []
    x = np.asarray(x, f32)
    for cid in range(NCORES):
        xs = x[cid * BSH:(cid + 1) * BSH]          # [256, 1024]
        xT = np.ascontiguousarray(
            xs.T.reshape(KI, 128, BSH).transpose(1, 0, 2).astype(f16))
        in_maps.append({"xT": xT, **shared})
    return in_maps


def kernel(x, W_in, b_in, W_ih, b_ih, W_hh, b_hh, W_out, b_out, trace=False):
    global LAST_EXEC_NS
    nc = _get_program()
    in_maps = _pack_inputs(x, W_in, b_in, W_ih, b_ih, W_hh, b_hh, W_out, b_out)
    if trace:
        trace = _install_ntff_hook()
    res = run_bass_kernel_spmd(nc, in_maps, core_ids=list(range(NCORES)), trace=trace)
    LAST_EXEC_NS = res.exec_time_ns
    return np.concatenate([res.results[c]["out"] for c in range(NCORES)], axis=0)


# revision 6
# speedup vs baseline: 1.0170x; 1.0170x over previous
"""Trainium2 Bass kernel for nn_LstmClassifier: batch-sharded LSTM over 8 cores.

fp8 (e4m3) DoubleRow recurrence:
    h is quantized to q = fp8(16*h) each step; gates = Wc8 . q via DoubleRow
    matmuls (2 k-tiles per instruction, 2x fp16 throughput). PSUM accumulates
    16*(Wc.h + bc); bias rides as a "ones-pair": q-tiles 4:6 hold 1.0 in
    partition 0, and Wc8 k-tiles 4:5 hold fp8(16*bc) hi/lo rows. The ACT
    engine applies sigmoid/tanh with scale=1/16 on whole gate tensors
    [128, 4, 256] (gate-major PSUM layout), so no per-partition bias fusion
    is needed. Step 0 additionally applies an fp8 lo-correction (q0lo) since
    h0 = relu(x.Win) is large and single-fp8 would inject too much noise.
    Cell math and the fp8 quantize run on DVE; tanh(c) on ACT; out-proj stays
    fp16 (exact) with bout added during the PSUM->SBUF eviction.
"""
import sys
import types
import numpy as np
import ml_dtypes

sys.path.insert(0, "/opt/trn_rl_repo")

import concourse.bass as bass  # noqa: E402
import concourse.tile as tile  # noqa: E402
from concourse import bacc, mybir  # noqa: E402
from concourse.bass_utils import run_bass_kernel_spmd  # noqa: E402

B, IN_DIM, HID, OUT_DIM, T = 2048, 1024, 512, 256, 64
NCORES = 8
BSH = B // NCORES          # 256 batch rows per core
KH = HID // 128            # 4 hidden k-tiles
KI = IN_DIM // 128         # 8 input k-tiles
F32 = mybir.dt.float32
F16 = mybir.dt.float16
F8 = mybir.dt.float8e4
AF = mybir.ActivationFunctionType
ALU = mybir.AluOpType
DR = mybir.MatmulPerfMode.DoubleRow
NPF8 = ml_dtypes.float8_e4m3fn

LAST_EXEC_NS = None


def _install_ntff_hook():
    try:
        import antenv.axon_hooks  # noqa: F401
        return True
    except ImportError:
        pass
    try:
        if "/root/.axon_site" not in sys.path:
            sys.path.insert(0, "/root/.axon_site")
        from trn_agent_boot.trn_boot import _ntff_profile_via_ctypes
        hook = _ntff_profile_via_ctypes("/opt/axon/libaxon_pjrt.so")
        if hook is None:
            return False
        import antenv
        mod = types.ModuleType("antenv.axon_hooks")
        mod._hook = hook
        mod.get_axon_ntff_profile_hook = lambda: mod._hook
        mod.set_axon_ntff_profile_hook = lambda h: setattr(mod, "_hook", h)
        antenv.axon_hooks = mod
        sys.modules["antenv.axon_hooks"] = mod
        return True
    except Exception:
        return False


def build_program(steps=T):
    nc = bacc.Bacc("TRN2", target_bir_lowering=False, debug=False)

    xT_d = nc.dram_tensor("xT", [128, KI, BSH], F16, kind="ExternalInput").ap()
    winT_d = nc.dram_tensor("winT", [128, KI, HID], F16, kind="ExternalInput").ap()
    bin_d = nc.dram_tensor("bin", [128, KH], F32, kind="ExternalInput").ap()
    # k-tiles 0:4 = fp8(Wc_hi) columns gate-major; 4:6 = bias hi/lo rows (x16)
    wc8_d = nc.dram_tensor("wc8", [128, 6, 4 * HID], F8, kind="ExternalInput").ap()
    wc8lo_d = nc.dram_tensor("wc8lo", [128, KH, 4 * HID], F8, kind="ExternalInput").ap()
    woutT_d = nc.dram_tensor("woutT", [128, KH, OUT_DIM], F16, kind="ExternalInput").ap()
    bout_d = nc.dram_tensor("bout", [128, OUT_DIM], F32, kind="ExternalInput").ap()
    out_d = nc.dram_tensor("out", [BSH, steps, OUT_DIM], F32, kind="ExternalOutput").ap()
    # [p, m, t, o]: batch row = m*128 + p
    out_v = out_d.rearrange("(m p) t o -> p m t o", p=128)

    from contextlib import ExitStack
    with tile.TileContext(nc) as tc, ExitStack() as ctx:
        consts = ctx.enter_context(tc.tile_pool(name="consts", bufs=1))
        qpool = ctx.enter_context(tc.tile_pool(name="q", bufs=3))
        qdpool = ctx.enter_context(tc.tile_pool(name="qd", bufs=2))
        hpool = ctx.enter_context(tc.tile_pool(name="h", bufs=3))
        cpool = ctx.enter_context(tc.tile_pool(name="c", bufs=3))
        sigp = ctx.enter_context(tc.tile_pool(name="sigs", bufs=2))
        tcp = ctx.enter_context(tc.tile_pool(name="tcp", bufs=2))
        osbp = ctx.enter_context(tc.tile_pool(name="osb", bufs=3))
        pg_pool = ctx.enter_context(tc.tile_pool(name="pgates", bufs=3, space="PSUM"))
        po_pool = ctx.enter_context(tc.tile_pool(name="pout", bufs=2, space="PSUM"))

        # ---- load constants ----
        xT = consts.tile([128, KI, BSH], F16)
        nc.sync.dma_start(xT[:], xT_d[:])
        winT = consts.tile([128, KI, HID], F16)
        nc.sync.dma_start(winT[:], winT_d[:])
        bin_sb = consts.tile([128, KH], F32)
        nc.sync.dma_start(bin_sb[:], bin_d[:])
        wc8 = consts.tile([128, 6, 4 * HID], F8)
        nc.sync.dma_start(wc8[:], wc8_d[:])
        wc8lo = consts.tile([128, KH, 4 * HID], F8)
        nc.sync.dma_start(wc8lo[:], wc8lo_d[:])
        woutT = consts.tile([128, KH, OUT_DIM], F16)
        nc.sync.dma_start(woutT[:], woutT_d[:])
        bout_sb = consts.tile([128, OUT_DIM], F32)
        nc.sync.dma_start(bout_sb[:], bout_d[:])
        bout_b = bout_sb[:].unsqueeze(1).broadcast_to([128, 2, OUT_DIM])

        # q buffers: [128, 6, BSH] fp8; tiles 4:6 are the bias "ones-pair"
        # (1.0 in partition 0, 0 elsewhere), preset once per rotating buffer.
        qtiles_init = []
        for r in range(3):
            qt = qpool.tile([128, 6, BSH], F8, tag="q", name=f"qinit{r}")
            nc.vector.memset(qt[:, 4:6, :], 0.0)
            nc.vector.memset(qt[0:1, 4:6, :], 1.0)
            qtiles_init.append(qt)

        # ---- input projection: h0 = relu(W_in @ x.T + b_in) (j-major) ----
        h = hpool.tile([128, KH, BSH], F16, tag="h")
        ph0 = pg_pool.tile([128, KH, BSH], F32, tag="pg", name="ph0")
        for jp in range(2):
            for k in range(KI):
                for j in (2 * jp, 2 * jp + 1):
                    nc.tensor.matmul(
                        ph0[:, j, :],
                        winT[:, k, j * 128:(j + 1) * 128],
                        xT[:, k, :],
                        start=(k == 0 and j % 2 == 0),
                        stop=(k == KI - 1 and j % 2 == 1),
                    )
        for j in range(KH):
            nc.scalar.activation(
                h[:, j, :], ph0[:, j, :], AF.Relu, bias=bin_sb[:, j:j + 1]
            )

        # q0 = fp8(16*h0); q0lo = fp8(16*(h0 - q0/16)) (step-0 lo correction)
        q = qtiles_init[0]
        nc.vector.tensor_scalar_mul(q[:, 0:4, :], h[:], 16.0)
        negr0 = consts.tile([128, KH, BSH], F16)
        nc.vector.scalar_tensor_tensor(
            negr0[:], q[:, 0:4, :], 0.0625, h[:], ALU.mult, ALU.subtract)
        q0lo = consts.tile([128, KH, BSH], F8)
        nc.vector.tensor_scalar_mul(q0lo[:], negr0[:], -16.0)
        qd = qdpool.tile([128, KH, BSH], F8, tag="qd", name="qd0")
        nc.vector.tensor_scalar_mul(qd[:], h[:], 1.0)

        WLO_FIRST = 10
        qinit_idx = 1

        def emit_gate_mms(pg, G, qt, extras=()):
            # one accumulation group per PSUM bank (j-pair)
            # extras: list of (lhsT_full, rhs_full) fp8 pairs, contracted over
            # k-tiles 0:4 in kp pairs
            for jp in range(2):
                mms = []
                for kp in range(3):  # kp 2 = ones/bias pair
                    for j in (2 * jp, 2 * jp + 1):
                        mms.append((None, kp, j))
                for ei, (lhs_t, rhs_t) in enumerate(extras):
                    for kp in range(2):
                        for j in (2 * jp, 2 * jp + 1):
                            mms.append((ei, kp, j))
                for idx, (ei, kp, j) in enumerate(mms):
                    mt = G * 4 + j
                    if ei is None:
                        lhs = wc8[:, 2 * kp:2 * kp + 2, mt * 128:(mt + 1) * 128]
                        rhs = qt[:, 2 * kp:2 * kp + 2, :]
                    else:
                        lhs_t, rhs_t = extras[ei]
                        lhs = lhs_t[:, 2 * kp:2 * kp + 2, mt * 128:(mt + 1) * 128]
                        rhs = rhs_t[:, 2 * kp:2 * kp + 2, :]
                    nc.tensor.matmul(
                        pg[:, j, :], lhs, rhs,
                        start=(idx == 0), stop=(idx == len(mms) - 1),
                        perf_mode=DR,
                    )

        def emit_outproj(h_src, t_idx):
            po = po_pool.tile([128, 2, OUT_DIM], F32, tag="po", name=f"po{t_idx}")
            for k in range(KH):
                for m in range(2):
                    nc.tensor.matmul(
                        po[:, m, :],
                        h_src[:, k, m * 128:(m + 1) * 128],
                        woutT[:, k, :],
                        start=(k == 0 and m == 0),
                        stop=(k == KH - 1 and m == 1),
                    )
            osb = osbp.tile([128, 2, OUT_DIM], F32, tag="osb", name=f"osb{t_idx}")
            nc.vector.tensor_add(osb[:], po[:], bout_b)
            nc.sync.dma_start(out_v[:, :, t_idx, :], osb[:])

        c = None
        for t in range(steps):
            extras = []
            if t == 0:
                extras.append((wc8, q0lo))
            if t < WLO_FIRST:
                extras.append((wc8lo, qd))
            # ---- gates (gate-major): f, i, g, o ----
            # ACT order: f, i, g01, o01, tanh01, g23, o23, tanh23 -- the j01
            # chain closes early so next step's kp0 matmuls start ~2us sooner.
            pgs = {}
            sigs = {}
            for G in (1, 0):  # f, i
                pg = pg_pool.tile([128, KH, BSH], F32, tag="pg", name=f"pg{t}_{G}")
                emit_gate_mms(pg, G, q, extras=extras)
                pgs[G] = pg
                if G == 1 and t >= 1:
                    emit_outproj(h, t - 1)
                sg = sigp.tile([128, KH, BSH], F16, tag=f"s{G}", name=f"s{t}_{G}")
                nc.scalar.activation(sg[:], pg[:], AF.Sigmoid, scale=0.0625)
                sigs[G] = sg
            for G in (2, 3):  # g, o
                pg = pg_pool.tile([128, KH, BSH], F32, tag="pg", name=f"pg{t}_{G}")
                emit_gate_mms(pg, G, q, extras=extras)
                pgs[G] = pg
                sg = sigp.tile([128, KH, BSH], F16, tag=f"s{G}", name=f"s{t}_{G}")
                sigs[G] = sg
            si, sf, tg, so = sigs[0], sigs[1], sigs[2], sigs[3]

            c_new = cpool.tile([128, KH, BSH], F16, tag="c")
            t1 = sigp.tile([128, KH, BSH], F16, tag="t1", name=f"t1_{t}")
            tc_t = tcp.tile([128, KH, BSH], F16, tag="tc", name=f"tc{t}")
            h_new = hpool.tile([128, KH, BSH], F16, tag="h")
            q_new = (qtiles_init[qinit_idx] if qinit_idx is not None and t < 2
                     else qpool.tile([128, 6, BSH], F8, tag="q", name=f"q{t}"))
            if t < 2:
                qinit_idx = 2 if qinit_idx == 1 else None
            qd_new = None
            if t + 1 < WLO_FIRST:
                qd_new = qdpool.tile([128, KH, BSH], F8, tag="qd", name=f"qd{t+1}")

            # cmul for both pairs right after ACT-f (c from prev step ready)
            if t >= 1:
                for jp in range(2):
                    s = slice(2 * jp, 2 * jp + 2)
                    nc.vector.tensor_mul(c_new[:, s, :], sf[:, s, :], c[:, s, :])

            for jp in range(2):
                s = slice(2 * jp, 2 * jp + 2)
                # ACT: g then o for this j-pair
                nc.scalar.activation(tg[:, s, :], pgs[2][:, s, :], AF.Tanh, scale=0.0625)
                nc.scalar.activation(so[:, s, :], pgs[3][:, s, :], AF.Sigmoid, scale=0.0625)
                # DVE chain for this pair
                nc.vector.tensor_mul(t1[:, s, :], si[:, s, :], tg[:, s, :])
                if t == 0:
                    nc.vector.tensor_copy(c_new[:, s, :], t1[:, s, :])
                else:
                    nc.vector.tensor_add(c_new[:, s, :], c_new[:, s, :], t1[:, s, :])
                nc.scalar.activation(tc_t[:, s, :], c_new[:, s, :], AF.Tanh)
                nc.vector.tensor_mul(h_new[:, s, :], so[:, s, :], tc_t[:, s, :])
                nc.vector.tensor_scalar_mul(q_new[:, s, :], h_new[:, s, :], 16.0)
                if qd_new is not None:
                    nc.vector.tensor_scalar_mul(qd_new[:, s, :], h_new[:, s, :], 1.0)

            if qd_new is not None:
                qd = qd_new
            h = h_new
            c = c_new
            q = q_new
        emit_outproj(h, steps - 1)

    nc.compile()
    return nc


_PROGRAM = None


def _get_program():
    global _PROGRAM
    if _PROGRAM is None:
        _PROGRAM = build_program()
    return _PROGRAM


def _pack_inputs(x, W_in, b_in, W_ih, b_ih, W_hh, b_hh, W_out, b_out):
    f16, f32 = np.float16, np.float32
    Wc = np.asarray(W_ih, f32) + np.asarray(W_hh, f32)
    bc = (np.asarray(b_ih, f32) + np.asarray(b_hh, f32))
    # gate-major m-tiles: mt = G*4 + j, G in (i, f, g, o) = PyTorch row order
    perm = np.concatenate([
        np.arange(G * HID + j * 128, G * HID + j * 128 + 128)
        for G in range(4) for j in range(KH)
    ])
    Wc_r = Wc[perm]                      # [2048, 512]
    bc_r = bc[perm]                      # [2048]

    # wc8: [128, 6, 2048]: k-tiles 0:4 = fp8(Wc_r.T), 4:6 = bias hi/lo rows
    wc8 = np.zeros((128, 6, 4 * HID), NPF8)
    wc8[:, 0:4, :] = np.ascontiguousarray(
        Wc_r.T.reshape(KH, 128, 4 * HID).transpose(1, 0, 2)).astype(NPF8)
    bhi8 = (16.0 * bc_r).astype(NPF8)
    bres = 16.0 * bc_r - bhi8.astype(f32)
    wc8[0, 4, :] = bhi8
    wc8[0, 5, :] = bres.astype(NPF8)
    # wc8lo: fp8(16*(Wc - fp8(Wc))), contracted against qd = fp8(h) (scale 1)
    wcT_f32 = np.ascontiguousarray(
        Wc_r.T.reshape(KH, 128, 4 * HID).transpose(1, 0, 2))
    wc8lo = (16.0 * (wcT_f32 - wc8[:, 0:4, :].astype(f32))).astype(NPF8)

    winT = np.ascontiguousarray(
        np.asarray(W_in, f32).T.reshape(KI, 128, HID).transpose(1, 0, 2).astype(f16))
    woutT = np.ascontiguousarray(
        np.asarray(W_out, f32).T.reshape(KH, 128, OUT_DIM).transpose(1, 0, 2).astype(f16))
    bin_p = np.ascontiguousarray(np.asarray(b_in, f32).reshape(KH, 128).T)
    bout_p = np.ascontiguousarray(np.broadcast_to(np.asarray(b_out, f32), (128, OUT_DIM)))

    shared = {
        "winT": winT, "wc8": wc8, "wc8lo": wc8lo, "woutT": woutT,
        "bin": bin_p, "bout": bout_p,
    }
    in_maps = []
    x = np.asarray(x, f32)
    for cid in range(NCORES):
        xs = x[cid * BSH:(cid + 1) * BSH]          # [256, 1024]
        xT = np.ascontiguousarray(
            xs.T.reshape(KI, 128, BSH).transpose(1, 0, 2).astype(f16))
        in_maps.append({"xT": xT, **shared})
    return in_maps


def kernel(x, W_in, b_in, W_ih, b_ih, W_hh, b_hh, W_out, b_out, trace=False):
    global LAST_EXEC_NS
    nc = _get_program()
    in_maps = _pack_inputs(x, W_in, b_in, W_ih, b_ih, W_hh, b_hh, W_out, b_out)
    if trace:
        trace = _install_ntff_hook()
    res = run_bass_kernel_spmd(nc, in_maps, core_ids=list(range(NCORES)), trace=trace)
    LAST_EXEC_NS = res.exec_time_ns
    return np.concatenate([res.results[c]["out"] for c in range(NCORES)], axis=0)


# revision 8
# speedup vs baseline: 1.2062x; 1.1860x over previous
"""Trainium2 Bass kernel for nn_LstmClassifier: batch-sharded LSTM over 8 cores.

Reference math (per batch row):
    h0 = relu(x @ W_in.T + b_in); c0 = 0
    64 steps of: gates = h @ (W_ih + W_hh).T + (b_ih + b_hh)   # input == hidden
                 i,f,g,o = split(gates); c = sig(f)*c + sig(i)*tanh(g); h = sig(o)*tanh(c)
    out[:, t, :] = h_t @ W_out.T + b_out

Device layout is hidden-major ("transposed"): h.T packed as [128 part, 4, 256],
so gate-unit biases are per-partition and fuse into the activation instructions,
and no transposes are needed anywhere in the recurrence.
"""
import sys
import types
import numpy as np

sys.path.insert(0, "/opt/trn_rl_repo")

import concourse.bass as bass  # noqa: E402
import concourse.tile as tile  # noqa: E402
from concourse import bacc, mybir  # noqa: E402
from concourse.bass_utils import run_bass_kernel_spmd  # noqa: E402

B, IN_DIM, HID, OUT_DIM, T = 2048, 1024, 512, 256, 64
NCORES = 8
BSH = B // NCORES          # 256 batch rows per core
KH = HID // 128            # 4 hidden k-tiles
KI = IN_DIM // 128         # 8 input k-tiles
NMT = 4 * HID // 128       # 16 gate m-tiles
F32 = mybir.dt.float32
F16 = mybir.dt.float16
AF = mybir.ActivationFunctionType

LAST_EXEC_NS = None


def _install_ntff_hook():
    try:
        import antenv.axon_hooks  # noqa: F401
        return True
    except ImportError:
        pass
    try:
        if "/root/.axon_site" not in sys.path:
            sys.path.insert(0, "/root/.axon_site")
        from trn_agent_boot.trn_boot import _ntff_profile_via_ctypes
        hook = _ntff_profile_via_ctypes("/opt/axon/libaxon_pjrt.so")
        if hook is None:
            return False
        import antenv
        mod = types.ModuleType("antenv.axon_hooks")
        mod._hook = hook
        mod.get_axon_ntff_profile_hook = lambda: mod._hook
        mod.set_axon_ntff_profile_hook = lambda h: setattr(mod, "_hook", h)
        antenv.axon_hooks = mod
        sys.modules["antenv.axon_hooks"] = mod
        return True
    except Exception:
        return False


def build_program(steps=T):
    nc = bacc.Bacc("TRN2", target_bir_lowering=False, debug=False)

    xT_d = nc.dram_tensor("xT", [128, KI, BSH], F16, kind="ExternalInput").ap()
    winT_d = nc.dram_tensor("winT", [128, KI, HID], F16, kind="ExternalInput").ap()
    wcT_d = nc.dram_tensor("wcT", [128, KH, 4 * HID], F16, kind="ExternalInput").ap()
    woutT_d = nc.dram_tensor("woutT", [128, KH, OUT_DIM], F16, kind="ExternalInput").ap()
    bin_d = nc.dram_tensor("bin", [128, KH], F32, kind="ExternalInput").ap()
    bc_d = nc.dram_tensor("bc", [128, NMT], F32, kind="ExternalInput").ap()
    bout_d = nc.dram_tensor("bout", [128, OUT_DIM], F32, kind="ExternalInput").ap()
    out_d = nc.dram_tensor("out", [BSH, steps, OUT_DIM], F32, kind="ExternalOutput").ap()
    # [p, m, t, o]: batch row = m*128 + p
    out_v = out_d.rearrange("(m p) t o -> p m t o", p=128)

    from contextlib import ExitStack
    with tile.TileContext(nc) as tc, ExitStack() as ctx:
        consts = ctx.enter_context(tc.tile_pool(name="consts", bufs=1))
        hpool = ctx.enter_context(tc.tile_pool(name="h", bufs=4))
        cpool = ctx.enter_context(tc.tile_pool(name="c", bufs=3))
        actp = ctx.enter_context(tc.tile_pool(name="acts", bufs=6))
        dvep = ctx.enter_context(tc.tile_pool(name="dvet", bufs=4))
        osbp = ctx.enter_context(tc.tile_pool(name="osb", bufs=4))
        pg_pool = ctx.enter_context(tc.tile_pool(name="pgates", bufs=4, space="PSUM"))

        # ---- load constants ----
        xT = consts.tile([128, KI, BSH], F16)
        nc.sync.dma_start(xT[:], xT_d[:])
        winT = consts.tile([128, KI, HID], F16)
        nc.sync.dma_start(winT[:], winT_d[:])
        wcT = consts.tile([128, KH, 4 * HID], F16)
        nc.sync.dma_start(wcT[:], wcT_d[:])
        woutT = consts.tile([128, KH, OUT_DIM], F16)
        nc.sync.dma_start(woutT[:], woutT_d[:])
        bin_sb = consts.tile([128, KH], F32)
        nc.sync.dma_start(bin_sb[:], bin_d[:])
        bc_sb = consts.tile([128, NMT], F32)
        nc.sync.dma_start(bc_sb[:], bc_d[:])
        bout_sb = consts.tile([128, OUT_DIM], F32)
        nc.sync.dma_start(bout_sb[:], bout_d[:])

        # ---- input projection: h0.T = relu(W_in @ x.T + b_in) ----
        h = hpool.tile([128, KH, BSH], F16, tag="h")
        ph0 = pg_pool.tile([128, 4, BSH], F32, tag="pg")
        for m in range(KH):
            for k in range(KI):
                nc.tensor.matmul(
                    ph0[:, m, :],
                    winT[:, k, m * 128:(m + 1) * 128],
                    xT[:, k, :],
                    start=(k == 0 and m % 2 == 0),
                    stop=(k == KI - 1 and m % 2 == 1),
                )
        for m in range(KH):
            nc.scalar.activation(
                h[:, m, :], ph0[:, m, :], AF.Relu, bias=bin_sb[:, m:m + 1]
            )

        bout_b = bout_sb[:].unsqueeze(1).broadcast_to([128, 2, OUT_DIM])

        def emit_outproj_mms(po, h_src, ks):
            # po is one PSUM bank: a single accumulation group (start on first
            # matmul touching the bank, stop on the last; unwritten bytes of a
            # started zero-region read as 0, so m=1's k=0 matmul may accumulate)
            for k in ks:
                for m in range(2):
                    nc.tensor.matmul(
                        po[:, m, :],
                        h_src[:, k, m * 128:(m + 1) * 128],
                        woutT[:, k, :],
                        start=(k == 0 and m == 0),
                        stop=(k == KH - 1 and m == 1),
                    )

        def emit_outproj_tail(po, t_idx):
            osb = osbp.tile([128, 2, OUT_DIM], F32, tag="osb", name=f"osb{t_idx}")
            nc.vector.tensor_add(osb[:], po[:], bout_b)
            nc.sync.dma_start(out_v[:, :, t_idx, :], osb[:])

        def emit_outproj(h_src, t_idx):
            pot = pg_pool.tile([128, 4, BSH], F32, tag="pg", name=f"po{t_idx}")
            po = pot[:, 0:2, :]
            emit_outproj_mms(po, h_src, range(KH))
            emit_outproj_tail(po, t_idx)

        c = None
        for t in range(steps):
            h_new = hpool.tile([128, KH, BSH], F16, tag="h")
            c_new = cpool.tile([128, KH, BSH], F16, tag="c")
            wave = [None] * KH  # per wave: (sig_i, sig_f, sig_o, tng)
            pgs = [None] * KH

            def mm_block(j, ks):
                # pg spans 2 banks (gi 0,1 | gi 2,3): one accumulation group
                # per bank — start on the bank's first matmul, stop on its last
                for k in ks:
                    for gi in range(4):
                        mt = 4 * j + gi
                        nc.tensor.matmul(
                            pgs[j][:, gi, :],
                            wcT[:, k, mt * 128:(mt + 1) * 128],
                            h[:, k, :],
                            start=(k == 0 and gi % 2 == 0),
                            stop=(k == KH - 1 and gi % 2 == 1),
                        )

            def act_block(j):
                pg = pgs[j]
                sig_i = actp.tile([128, BSH], F16, tag="sig_i", name=f"si{t}_{j}")
                sig_f = actp.tile([128, BSH], F16, tag="sig_f", name=f"sf{t}_{j}")
                sig_o = actp.tile([128, BSH], F16, tag="sig_o", name=f"so{t}_{j}")
                tng = actp.tile([128, BSH], F16, tag="tng", name=f"tg{t}_{j}")
                nc.scalar.activation(sig_i[:], pg[:, 0, :], AF.Sigmoid, bias=bc_sb[:, 4 * j + 0:4 * j + 1])
                nc.scalar.activation(sig_f[:], pg[:, 1, :], AF.Sigmoid, bias=bc_sb[:, 4 * j + 1:4 * j + 2])
                nc.scalar.activation(tng[:], pg[:, 3, :], AF.Tanh, bias=bc_sb[:, 4 * j + 3:4 * j + 4])
                nc.scalar.activation(sig_o[:], pg[:, 2, :], AF.Sigmoid, bias=bc_sb[:, 4 * j + 2:4 * j + 3])
                wave[j] = (sig_i, sig_f, sig_o, tng)

            def cell_update(j):
                # c_new[j] = sig(f)*c[j] + sig(i)*tanh(g)
                sig_i, sig_f, sig_o, tng = wave[j]
                if t == 0:
                    # c0 == 0: c1 = sig(i) * tanh(g)
                    nc.vector.tensor_mul(c_new[:, j, :], sig_i[:], tng[:])
                else:
                    t1 = dvep.tile([128, BSH], F16, tag="t1", name=f"t1_{t}_{j}")
                    nc.vector.tensor_mul(t1[:], sig_i[:], tng[:])
                    nc.vector.tensor_mul(c_new[:, j, :], sig_f[:], c[:, j, :])
                    nc.vector.tensor_add(c_new[:, j, :], c_new[:, j, :], t1[:])

            def h_update(jpair):
                # tanh(c) merged over a wave pair, then h = sig(o) * tanh(c)
                tnc = dvep.tile([128, 2, BSH], F16, tag="tnc", name=f"tnc{t}_{jpair}")
                nc.scalar.activation(tnc[:], c_new[:, 2 * jpair:2 * jpair + 2, :], AF.Tanh)
                for j in (2 * jpair, 2 * jpair + 1):
                    nc.vector.tensor_mul(h_new[:, j, :], wave[j][2][:], tnc[:, j - 2 * jpair, :])

            def h_update_single(j, split=False):
                # last waves get individual tanh(c) so h[2] exits the
                # end-of-step dependency ring before h[3]'s chain completes;
                # h[3]'s multiply is split into batch halves so the next
                # step's k3 matmuls can start on the first half early
                tnc = dvep.tile([128, 2, BSH], F16, tag="tnc", name=f"tncs{t}_{j}")
                nc.scalar.activation(tnc[:, 0, :], c_new[:, j, :], AF.Tanh)
                if split:
                    hb = BSH // 2
                    nc.vector.tensor_mul(h_new[:, j, :hb], wave[j][2][:, :hb], tnc[:, 0, :hb])
                    nc.vector.tensor_mul(h_new[:, j, hb:], wave[j][2][:, hb:], tnc[:, 0, hb:])
                else:
                    nc.vector.tensor_mul(h_new[:, j, :], wave[j][2][:], tnc[:, 0, :])

            # PE stream: front-load k0-k2 matmuls of wave0/outproj/wave1 (they
            # need only early h slices) so the PE advances through the
            # end-of-step ring while the last wave's pointwise chain finishes;
            # the k3 blocks (gated on the final h slice) come right after.
            pgs[0] = pg_pool.tile([128, 4, BSH], F32, tag="pg", name=f"pg{t}_0")
            mm_block(0, range(KH - 1))
            po = None
            if t >= 1:
                pot = pg_pool.tile([128, 4, BSH], F32, tag="pg", name=f"po{t-1}")
                po = pot[:, 0:2, :]
                emit_outproj_mms(po, h, range(KH - 1))
            pgs[1] = pg_pool.tile([128, 4, BSH], F32, tag="pg", name=f"pg{t}_1")
            mm_block(1, range(KH - 1))
            # wave0's k3 matmuls run in batch halves: the a-half issues as soon
            # as the first half of h[3] lands, overlapping the rest of its chain
            # bank A (i,f — gating the first sigmoid of the ring) completes
            # before any bank B matmul issues
            for pair in range(2):
                for half in range(2):
                    lo, hi = half * (BSH // 2), (half + 1) * (BSH // 2)
                    for gi in (2 * pair, 2 * pair + 1):
                        mt = 4 * 0 + gi
                        nc.tensor.matmul(
                            pgs[0][:, gi, lo:hi],
                            wcT[:, KH - 1, mt * 128:(mt + 1) * 128],
                            h[:, KH - 1, lo:hi],
                            start=False,
                            stop=(half == 1 and gi % 2 == 1),
                        )
            act_block(0)
            mm_block(1, [KH - 1])
            act_block(1)
            cell_update(0)
            if t >= 1:
                emit_outproj_mms(po, h, [KH - 1])
                emit_outproj_tail(po, t - 1)
            pgs[2] = pg_pool.tile([128, 4, BSH], F32, tag="pg", name=f"pg{t}_2")
            mm_block(2, range(KH))
            act_block(2)
            cell_update(1)
            h_update(0)
            pgs[3] = pg_pool.tile([128, 4, BSH], F32, tag="pg", name=f"pg{t}_3")
            mm_block(3, range(KH))
            act_block(3)
            cell_update(2)
            h_update_single(2)
            cell_update(KH - 1)
            h_update_single(3, split=True)

            h = h_new
            c = c_new
        emit_outproj(h, steps - 1)

    nc.compile()
    return nc


_PROGRAM = None


def _get_program():
    global _PROGRAM
    if _PROGRAM is None:
        _PROGRAM = build_program()
    return _PROGRAM


def _pack_inputs(x, W_in, b_in, W_ih, b_ih, W_hh, b_hh, W_out, b_out):
    f16, f32 = np.float16, np.float32
    Wc = (np.asarray(W_ih, f32) + np.asarray(W_hh, f32))
    bc = (np.asarray(b_ih, f32) + np.asarray(b_hh, f32))
    # reorder gate rows to m-tiles [i_j, f_j, o_j, g_j] (PyTorch order i,f,g,o)
    base = {0: 0, 1: HID, 2: 3 * HID, 3: 2 * HID}  # gi -> original row block
    perm = np.concatenate([
        np.arange(base[gi] + j * 128, base[gi] + j * 128 + 128)
        for j in range(KH) for gi in range(4)
    ])
    Wc_r = Wc[perm]                      # [2048, 512]
    bc_r = bc[perm]                      # [2048]
    wcT = np.ascontiguousarray(
        Wc_r.T.reshape(KH, 128, 4 * HID).transpose(1, 0, 2).astype(f16))
    winT = np.ascontiguousarray(
        np.asarray(W_in, f32).T.reshape(KI, 128, HID).transpose(1, 0, 2).astype(f16))
    woutT = np.ascontiguousarray(
        np.asarray(W_out, f32).T.reshape(KH, 128, OUT_DIM).transpose(1, 0, 2).astype(f16))
    bin_p = np.ascontiguousarray(np.asarray(b_in, f32).reshape(KH, 128).T)
    bc_p = np.ascontiguousarray(bc_r.reshape(NMT, 128).T)
    bout_p = np.ascontiguousarray(np.broadcast_to(np.asarray(b_out, f32), (128, OUT_DIM)))

    shared = {
        "winT": winT, "wcT": wcT, "woutT": woutT,
        "bin": bin_p, "bc": bc_p, "bout": bout_p,
    }
    in_maps = []
    x = np.asarray(x, f32)
    for cid in range(NCORES):
        xs = x[cid * BSH:(cid + 1) * BSH]          # [256, 1024]
        xT = np.ascontiguousarray(
            xs.T.reshape(KI, 128, BSH).transpose(1, 0, 2).astype(f16))
        in_maps.append({"xT": xT, **shared})
    return in_maps


def kernel(x, W_in, b_in, W_ih, b_ih, W_hh, b_hh, W_out, b_out, trace=False):
    global LAST_EXEC_NS
    nc = _get_program()
    in_maps = _pack_inputs(x, W_in, b_in, W_ih, b_ih, W_hh, b_hh, W_out, b_out)
    if trace:
        trace = _install_ntff_hook()
    res = run_bass_kernel_spmd(nc, in_maps, core_ids=list(range(NCORES)), trace=trace)
    LAST_EXEC_NS = res.exec_time_ns
    return np.concatenate([res.results[c]["out"] for c in range(NCORES)], axis=0)



# revision 9
# speedup vs baseline: 1.2072x; 1.0008x over previous
"""Trainium2 Bass kernel for nn_LstmClassifier: batch-sharded LSTM over 8 cores.

Reference math (per batch row):
    h0 = relu(x @ W_in.T + b_in); c0 = 0
    64 steps of: gates = h @ (W_ih + W_hh).T + (b_ih + b_hh)   # input == hidden
                 i,f,g,o = split(gates); c = sig(f)*c + sig(i)*tanh(g); h = sig(o)*tanh(c)
    out[:, t, :] = h_t @ W_out.T + b_out

Device layout is hidden-major ("transposed"): h.T packed as [128 part, 4, 256],
so gate-unit biases are per-partition and fuse into the activation instructions,
and no transposes are needed anywhere in the recurrence.
"""
import sys
import types
import numpy as np

sys.path.insert(0, "/opt/trn_rl_repo")

import concourse.bass as bass  # noqa: E402
import concourse.tile as tile  # noqa: E402
from concourse import bacc, mybir  # noqa: E402
from concourse.bass_utils import run_bass_kernel_spmd  # noqa: E402

B, IN_DIM, HID, OUT_DIM, T = 2048, 1024, 512, 256, 64
NCORES = 8
BSH = B // NCORES          # 256 batch rows per core
KH = HID // 128            # 4 hidden k-tiles
KI = IN_DIM // 128         # 8 input k-tiles
NMT = 4 * HID // 128       # 16 gate m-tiles
F32 = mybir.dt.float32
F16 = mybir.dt.float16
AF = mybir.ActivationFunctionType

LAST_EXEC_NS = None


def _install_ntff_hook():
    try:
        import antenv.axon_hooks  # noqa: F401
        return True
    except ImportError:
        pass
    try:
        if "/root/.axon_site" not in sys.path:
            sys.path.insert(0, "/root/.axon_site")
        from trn_agent_boot.trn_boot import _ntff_profile_via_ctypes
        hook = _ntff_profile_via_ctypes("/opt/axon/libaxon_pjrt.so")
        if hook is None:
            return False
        import antenv
        mod = types.ModuleType("antenv.axon_hooks")
        mod._hook = hook
        mod.get_axon_ntff_profile_hook = lambda: mod._hook
        mod.set_axon_ntff_profile_hook = lambda h: setattr(mod, "_hook", h)
        antenv.axon_hooks = mod
        sys.modules["antenv.axon_hooks"] = mod
        return True
    except Exception:
        return False


def build_program(steps=T):
    nc = bacc.Bacc("TRN2", target_bir_lowering=False, debug=False)

    xT_d = nc.dram_tensor("xT", [128, KI, BSH], F16, kind="ExternalInput").ap()
    winT_d = nc.dram_tensor("winT", [128, KI, HID], F16, kind="ExternalInput").ap()
    wcT_d = nc.dram_tensor("wcT", [128, KH, 4 * HID], F16, kind="ExternalInput").ap()
    woutT_d = nc.dram_tensor("woutT", [128, KH, OUT_DIM], F16, kind="ExternalInput").ap()
    bin_d = nc.dram_tensor("bin", [128, KH], F32, kind="ExternalInput").ap()
    bc_d = nc.dram_tensor("bc", [128, NMT], F32, kind="ExternalInput").ap()
    bout_d = nc.dram_tensor("bout", [128, OUT_DIM], F32, kind="ExternalInput").ap()
    out_d = nc.dram_tensor("out", [BSH, steps, OUT_DIM], F32, kind="ExternalOutput").ap()
    # [p, m, t, o]: batch row = m*128 + p
    out_v = out_d.rearrange("(m p) t o -> p m t o", p=128)

    from contextlib import ExitStack
    with tile.TileContext(nc) as tc, ExitStack() as ctx:
        consts = ctx.enter_context(tc.tile_pool(name="consts", bufs=1))
        hpool = ctx.enter_context(tc.tile_pool(name="h", bufs=6))
        cpool = ctx.enter_context(tc.tile_pool(name="c", bufs=5))
        actp = ctx.enter_context(tc.tile_pool(name="acts", bufs=8))
        dvep = ctx.enter_context(tc.tile_pool(name="dvet", bufs=6))
        osbp = ctx.enter_context(tc.tile_pool(name="osb", bufs=8))
        pg_pool = ctx.enter_context(tc.tile_pool(name="pgates", bufs=4, space="PSUM"))

        # ---- load constants ----
        xT = consts.tile([128, KI, BSH], F16)
        nc.sync.dma_start(xT[:], xT_d[:])
        winT = consts.tile([128, KI, HID], F16)
        nc.sync.dma_start(winT[:], winT_d[:])
        wcT = consts.tile([128, KH, 4 * HID], F16)
        nc.sync.dma_start(wcT[:], wcT_d[:])
        woutT = consts.tile([128, KH, OUT_DIM], F16)
        nc.sync.dma_start(woutT[:], woutT_d[:])
        bin_sb = consts.tile([128, KH], F32)
        nc.sync.dma_start(bin_sb[:], bin_d[:])
        bc_sb = consts.tile([128, NMT], F32)
        nc.sync.dma_start(bc_sb[:], bc_d[:])
        bout_sb = consts.tile([128, OUT_DIM], F32)
        nc.sync.dma_start(bout_sb[:], bout_d[:])

        # ---- input projection: h0.T = relu(W_in @ x.T + b_in) ----
        h = hpool.tile([128, KH, BSH], F16, tag="h")
        ph0 = pg_pool.tile([128, 4, BSH], F32, tag="pg")
        for m in range(KH):
            for k in range(KI):
                nc.tensor.matmul(
                    ph0[:, m, :],
                    winT[:, k, m * 128:(m + 1) * 128],
                    xT[:, k, :],
                    start=(k == 0 and m % 2 == 0),
                    stop=(k == KI - 1 and m % 2 == 1),
                )
        for m in range(KH):
            nc.scalar.activation(
                h[:, m, :], ph0[:, m, :], AF.Relu, bias=bin_sb[:, m:m + 1]
            )

        bout_b = bout_sb[:].unsqueeze(1).broadcast_to([128, 2, OUT_DIM])

        def emit_outproj_mms(po, h_src, ks):
            # po is one PSUM bank: a single accumulation group (start on first
            # matmul touching the bank, stop on the last; unwritten bytes of a
            # started zero-region read as 0, so m=1's k=0 matmul may accumulate)
            for k in ks:
                for m in range(2):
                    nc.tensor.matmul(
                        po[:, m, :],
                        h_src[:, k, m * 128:(m + 1) * 128],
                        woutT[:, k, :],
                        start=(k == 0 and m == 0),
                        stop=(k == KH - 1 and m == 1),
                    )

        def emit_outproj_tail(po, t_idx):
            osb = osbp.tile([128, 2, OUT_DIM], F32, tag="osb", name=f"osb{t_idx}")
            nc.vector.tensor_add(osb[:], po[:], bout_b)
            nc.sync.dma_start(out_v[:, :, t_idx, :], osb[:])

        def emit_outproj(h_src, t_idx):
            pot = pg_pool.tile([128, 4, BSH], F32, tag="pg", name=f"po{t_idx}")
            po = pot[:, 0:2, :]
            emit_outproj_mms(po, h_src, range(KH))
            emit_outproj_tail(po, t_idx)

        c = None
        for t in range(steps):
            h_new = hpool.tile([128, KH, BSH], F16, tag="h")
            c_new = cpool.tile([128, KH, BSH], F16, tag="c")
            wave = [None] * KH  # per wave: (sig_i, sig_f, sig_o, tng)
            pgs = [None] * KH

            def mm_block(j, ks):
                # pg spans 2 banks (gi 0,1 | gi 2,3): one accumulation group
                # per bank — start on the bank's first matmul, stop on its last
                for k in ks:
                    for gi in range(4):
                        mt = 4 * j + gi
                        nc.tensor.matmul(
                            pgs[j][:, gi, :],
                            wcT[:, k, mt * 128:(mt + 1) * 128],
                            h[:, k, :],
                            start=(k == 0 and gi % 2 == 0),
                            stop=(k == KH - 1 and gi % 2 == 1),
                        )

            def act_block(j):
                pg = pgs[j]
                sig_i = actp.tile([128, BSH], F16, tag="sig_i", name=f"si{t}_{j}")
                sig_f = actp.tile([128, BSH], F16, tag="sig_f", name=f"sf{t}_{j}")
                sig_o = actp.tile([128, BSH], F16, tag="sig_o", name=f"so{t}_{j}")
                tng = actp.tile([128, BSH], F16, tag="tng", name=f"tg{t}_{j}")
                nc.scalar.activation(sig_i[:], pg[:, 0, :], AF.Sigmoid, bias=bc_sb[:, 4 * j + 0:4 * j + 1])
                nc.scalar.activation(sig_f[:], pg[:, 1, :], AF.Sigmoid, bias=bc_sb[:, 4 * j + 1:4 * j + 2])
                nc.scalar.activation(tng[:], pg[:, 3, :], AF.Tanh, bias=bc_sb[:, 4 * j + 3:4 * j + 4])
                nc.scalar.activation(sig_o[:], pg[:, 2, :], AF.Sigmoid, bias=bc_sb[:, 4 * j + 2:4 * j + 3])
                wave[j] = (sig_i, sig_f, sig_o, tng)

            def cell_update(j):
                # c_new[j] = sig(f)*c[j] + sig(i)*tanh(g)
                sig_i, sig_f, sig_o, tng = wave[j]
                if t == 0:
                    # c0 == 0: c1 = sig(i) * tanh(g)
                    nc.vector.tensor_mul(c_new[:, j, :], sig_i[:], tng[:])
                else:
                    t1 = dvep.tile([128, BSH], F16, tag="t1", name=f"t1_{t}_{j}")
                    nc.vector.tensor_mul(t1[:], sig_i[:], tng[:])
                    nc.vector.tensor_mul(c_new[:, j, :], sig_f[:], c[:, j, :])
                    nc.vector.tensor_add(c_new[:, j, :], c_new[:, j, :], t1[:])

            def h_update(jpair):
                # tanh(c) merged over a wave pair, then h = sig(o) * tanh(c)
                tnc = dvep.tile([128, 2, BSH], F16, tag="tnc", name=f"tnc{t}_{jpair}")
                nc.scalar.activation(tnc[:], c_new[:, 2 * jpair:2 * jpair + 2, :], AF.Tanh)
                for j in (2 * jpair, 2 * jpair + 1):
                    nc.vector.tensor_mul(h_new[:, j, :], wave[j][2][:], tnc[:, j - 2 * jpair, :])

            def h_update_single(j, split=False):
                # last waves get individual tanh(c) so h[2] exits the
                # end-of-step dependency ring before h[3]'s chain completes;
                # h[3]'s multiply is split into batch halves so the next
                # step's k3 matmuls can start on the first half early
                tnc = dvep.tile([128, 2, BSH], F16, tag="tnc", name=f"tncs{t}_{j}")
                nc.scalar.activation(tnc[:, 0, :], c_new[:, j, :], AF.Tanh)
                if split:
                    hb = BSH // 2
                    nc.vector.tensor_mul(h_new[:, j, :hb], wave[j][2][:, :hb], tnc[:, 0, :hb])
                    nc.vector.tensor_mul(h_new[:, j, hb:], wave[j][2][:, hb:], tnc[:, 0, hb:])
                else:
                    nc.vector.tensor_mul(h_new[:, j, :], wave[j][2][:], tnc[:, 0, :])

            # PE stream: front-load k0-k2 matmuls of wave0/outproj/wave1 (they
            # need only early h slices) so the PE advances through the
            # end-of-step ring while the last wave's pointwise chain finishes;
            # the k3 blocks (gated on the final h slice) come right after.
            pgs[0] = pg_pool.tile([128, 4, BSH], F32, tag="pg", name=f"pg{t}_0")
            mm_block(0, range(KH - 1))
            po = None
            if t >= 1:
                pot = pg_pool.tile([128, 4, BSH], F32, tag="pg", name=f"po{t-1}")
                po = pot[:, 0:2, :]
                emit_outproj_mms(po, h, range(KH - 1))
            pgs[1] = pg_pool.tile([128, 4, BSH], F32, tag="pg", name=f"pg{t}_1")
            mm_block(1, range(KH - 1))
            # wave0's k3 matmuls run in batch halves: the a-half issues as soon
            # as the first half of h[3] lands, overlapping the rest of its chain
            # bank A (i,f — gating the first sigmoid of the ring) completes
            # before any bank B matmul issues
            for pair in range(2):
                for half in range(2):
                    lo, hi = half * (BSH // 2), (half + 1) * (BSH // 2)
                    for gi in (2 * pair, 2 * pair + 1):
                        mt = 4 * 0 + gi
                        nc.tensor.matmul(
                            pgs[0][:, gi, lo:hi],
                            wcT[:, KH - 1, mt * 128:(mt + 1) * 128],
                            h[:, KH - 1, lo:hi],
                            start=False,
                            stop=(half == 1 and gi % 2 == 1),
                        )
            act_block(0)
            mm_block(1, [KH - 1])
            act_block(1)
            cell_update(0)
            if t >= 1:
                emit_outproj_mms(po, h, [KH - 1])
                emit_outproj_tail(po, t - 1)
            pgs[2] = pg_pool.tile([128, 4, BSH], F32, tag="pg", name=f"pg{t}_2")
            mm_block(2, range(KH))
            act_block(2)
            cell_update(1)
            h_update(0)
            pgs[3] = pg_pool.tile([128, 4, BSH], F32, tag="pg", name=f"pg{t}_3")
            mm_block(3, range(KH))
            act_block(3)
            cell_update(2)
            h_update_single(2)
            cell_update(KH - 1)
            h_update_single(3, split=True)

            h = h_new
            c = c_new
        emit_outproj(h, steps - 1)

    nc.compile()
    return nc


_PROGRAM = None


def _get_program():
    global _PROGRAM
    if _PROGRAM is None:
        _PROGRAM = build_program()
    return _PROGRAM


def _pack_inputs(x, W_in, b_in, W_ih, b_ih, W_hh, b_hh, W_out, b_out):
    f16, f32 = np.float16, np.float32
    Wc = (np.asarray(W_ih, f32) + np.asarray(W_hh, f32))
    bc = (np.asarray(b_ih, f32) + np.asarray(b_hh, f32))
    # reorder gate rows to m-tiles [i_j, f_j, o_j, g_j] (PyTorch order i,f,g,o)
    base = {0: 0, 1: HID, 2: 3 * HID, 3: 2 * HID}  # gi -> original row block
    perm = np.concatenate([
        np.arange(base[gi] + j * 128, base[gi] + j * 128 + 128)
        for j in range(KH) for gi in range(4)
    ])
    Wc_r = Wc[perm]                      # [2048, 512]
    bc_r = bc[perm]                      # [2048]
    wcT = np.ascontiguousarray(
        Wc_r.T.reshape(KH, 128, 4 * HID).transpose(1, 0, 2).astype(f16))
    winT = np.ascontiguousarray(
        np.asarray(W_in, f32).T.reshape(KI, 128, HID).transpose(1, 0, 2).astype(f16))
    woutT = np.ascontiguousarray(
        np.asarray(W_out, f32).T.reshape(KH, 128, OUT_DIM).transpose(1, 0, 2).astype(f16))
    bin_p = np.ascontiguousarray(np.asarray(b_in, f32).reshape(KH, 128).T)
    bc_p = np.ascontiguousarray(bc_r.reshape(NMT, 128).T)
    bout_p = np.ascontiguousarray(np.broadcast_to(np.asarray(b_out, f32), (128, OUT_DIM)))

    shared = {
        "winT": winT, "wcT": wcT, "woutT": woutT,
        "bin": bin_p, "bc": bc_p, "bout": bout_p,
    }
    in_maps = []
    x = np.asarray(x, f32)
    for cid in range(NCORES):
        xs = x[cid * BSH:(cid + 1) * BSH]          # [256, 1024]
        xT = np.ascontiguousarray(
            xs.T.reshape(KI, 128, BSH).transpose(1, 0, 2).astype(f16))
        in_maps.append({"xT": xT, **shared})
    return in_maps


def kernel(x, W_in, b_in, W_ih, b_ih, W_hh, b_hh, W_out, b_out, trace=False):
    global LAST_EXEC_NS
    nc = _get_program()
    in_maps = _pack_inputs(x, W_in, b_in, W_ih, b_ih, W_hh, b_hh, W_out, b_out)
    if trace:
        trace = _install_ntff_hook()
    res = run_bass_kernel_spmd(nc, in_maps, core_ids=list(range(NCORES)), trace=trace)
    LAST_EXEC_NS = res.exec_time_ns
    return np.concatenate([res.results[c]["out"] for c in range(NCORES)], axis=0)

